# revision 1
# baseline (speedup 1.0000x reference)
"""Trainium2 Bass kernel for nn_C2SModel (code2seq-style model).

Self-contained: host-side sharding by sample across 8 NeuronCores, one SPMD
Bass/Tile program per core, host-side gather of per-core outputs.

Model (see problem statement):
  - segment-sum of gathered subtoken embeddings (ll/rl) per context
  - bidirectional LSTM over 9-node paths, final hidden states
  - ctx = tanh(fc([ll, h_f, h_b, rl]));  segment-softmax attention over
    contexts within each sample;  out = v @ out_w.T + out_b
"""
import numpy as np
import ml_dtypes
from contextlib import ExitStack

BF16 = ml_dtypes.bfloat16

# ---- problem constants (hardcoded per contract) ---------------------------
N_CTX = 65536
B = 512
E = 128
H = 128
T = 9
SUB_V = 50000
NODE_V = 512
DEC = 320
OUT_D = 10000
N_CORES = 8
SPC = B // N_CORES            # 64 samples per core
NP = 8704                     # padded contexts per core (68*128)
NT128 = NP // 128             # 68
NT512 = NP // 512             # 17
SPLIT = 32768                 # subtoken table split (int16 gather indices)
GROUP = 4                     # ctx-tiles of 128 per gather call group
NGRP = (NT128 + GROUP - 1) // GROUP
OUT_NCH = 500                 # final matmul N-chunk (20 chunks of 500)
SPANS = [(n, min(1024, NP - n)) for n in range(0, NP, 1024)]


# ---- host-side prep -------------------------------------------------------

def _wrap_idx(ids):
    ids = np.asarray(ids, np.int16)
    assert len(ids) % 16 == 0
    w16 = ids.reshape(-1, 16).T
    return np.tile(w16, (8, 1)).copy()


def _pad_to(x, n, val):
    out = np.full((n,) + x.shape[1:], val, x.dtype)
    out[: len(x)] = x
    return out


def _core_raw(k, inp):
    """Per-core raw (unpadded-block) occurrence lists and context data."""
    indices = inp["indices"]
    s = int(np.searchsorted(indices, k * SPC, "left"))
    e = int(np.searchsorted(indices, (k + 1) * SPC, "left"))
    nk = e - s
    assert nk <= NP, f"core {k}: {nk} contexts exceed NP={NP}"
    samp = _pad_to(indices[s:e].astype(np.int32) - k * SPC, NP, -1)
    pth = _pad_to(inp["paths"][s:e].astype(np.int16), NP, 0)
    d = {"samp": samp, "paths": pth, "nk": nk}
    for side in ("ll", "rl"):
        idxs_g = inp[f"{side}_indices"]
        subs_g = inp[f"{side}_subtokens"]
        o_s = int(np.searchsorted(idxs_g, s, "left"))
        o_e = int(np.searchsorted(idxs_g, e, "left"))
        subs = subs_g[o_s:o_e].astype(np.int32)
        ctxs = idxs_g[o_s:o_e].astype(np.int32) - s
        tb = np.searchsorted(ctxs, np.arange(0, NP + 128, 128))
        per_tile = {"lo": [], "hi": []}
        for t in range(NT128):
            sl = slice(tb[t], tb[t + 1])
            tsub, tctx = subs[sl], ctxs[sl] - t * 128
            m = tsub < SPLIT
            per_tile["lo"].append((tsub[m], tctx[m]))
            per_tile["hi"].append((tsub[~m] - SPLIT, tctx[~m]))
        d[side] = per_tile
    return d


def prep_all(inp):
    """Returns (meta, per_core_data). meta holds the uniform block structure."""
    raws = [_core_raw(k, inp) for k in range(N_CORES)]
    # uniform per-tile block counts (max over cores)
    nb = {}
    for side in ("ll", "rl"):
        for cls in ("lo", "hi"):
            nb[(side, cls)] = np.array(
                [
                    max(
                        (len(r[side][cls][t][0]) + 127) // 128
                        for r in raws
                    )
                    for t in range(NT128)
                ],
                np.int32,
            )
    meta = {"nb": nb}

    cores = []
    for r in raws:
        d = {"samp": r["samp"], "nk": r["nk"]}
        # x-gather stream: per 512-tile, t-major [9, 512]
        pth = r["paths"]
        xidx = np.concatenate(
            [pth[n0 : n0 + W].T.reshape(-1) for (n0, W) in SPANS]
        )
        d["xidx"] = _wrap_idx(xidx)
        d["sampid"] = (
            np.ascontiguousarray(r["samp"].reshape(NT128, 128).T.astype(np.float32))
        )
        for side in ("ll", "rl"):
            for cls in ("lo", "hi"):
                subs_s, ctxs_s = [], []
                for t in range(NT128):
                    ts_, tc_ = r[side][cls][t]
                    n = nb[(side, cls)][t] * 128
                    subs_s.append(_pad_to(ts_.astype(np.int32), n, 0))
                    ctxs_s.append(_pad_to(tc_.astype(np.int32), n, -1))
                subs_s = np.concatenate(subs_s) if subs_s else np.zeros(0, np.int32)
                ctxs_s = np.concatenate(ctxs_s) if ctxs_s else np.zeros(0, np.int32)
                d[f"{side}_{cls}_idx"] = _wrap_idx(subs_s.astype(np.int16))
                d[f"{side}_{cls}_ctx"] = np.ascontiguousarray(
                    ctxs_s.reshape(-1, 128).T.astype(np.float32)
                )  # [128, Btot]
        cores.append(d)
    return meta, cores


def prep_shared(inp):
    """Replicated (same for all cores) tensors."""
    sub = np.asarray(inp["subtoken_emb"], np.float32)
    node = np.asarray(inp["node_emb"], np.float32)
    sh = {
        "sub_lo": sub[:SPLIT].astype(BF16),
        "sub_hi": sub[SPLIT:].astype(BF16),
        "node_t": node.astype(BF16),
        "ramp": np.tile(np.arange(128, dtype=np.float32), (128, 1)).astype(BF16),
        "a_bc": np.tile(np.asarray(inp["a"], np.float32), (128, 1)).astype(BF16),
    }
    for d in ("f", "b"):
        sh[f"wih_{d}"] = np.ascontiguousarray(
            np.asarray(inp[f"w_ih_{d}"], np.float32).T
        ).astype(BF16)  # [128 e, 512 g]
        sh[f"whh_{d}"] = np.ascontiguousarray(
            np.asarray(inp[f"w_hh_{d}"], np.float32).T
        ).astype(BF16)  # [128 h, 512 g]
        bias = np.asarray(inp[f"b_ih_{d}"], np.float32) + np.asarray(
            inp[f"b_hh_{d}"], np.float32
        )
        sh[f"bias_{d}"] = np.ascontiguousarray(bias.reshape(4, 128).T)  # [128, 4] f32
    fcwT = np.asarray(inp["fc_w"], np.float32).T  # [512, 320]
    sh["fcw"] = np.ascontiguousarray(
        fcwT.reshape(4, 128, DEC).transpose(1, 0, 2).reshape(128, 4 * DEC)
    ).astype(BF16)  # [128, 4*320], chunk-major
    outw = np.concatenate(
        [np.asarray(inp["out_w"], np.float32).T,
         np.asarray(inp["out_b"], np.float32)[None, :]], axis=0
    )  # [321, 10000], row 320 = out_b
    sh["outw"] = outw.astype(BF16)
    return sh


# ---- bass program ---------------------------------------------------------

def build_nc(meta, shapes, phases=('A','B','C')):
    import concourse.bass as bass
    import concourse.bacc as bacc
    import concourse.tile as tile
    import concourse.mybir as mybir
    from concourse.library_config import mlp as mlp_lib

    dt = mybir.dt
    AF = mybir.ActivationFunctionType
    ALU = mybir.AluOpType
    nb = meta["nb"]

    nc = bacc.Bacc("TRN2", target_bir_lowering=False, debug=False,
                   num_devices=N_CORES)

    def din(name, shape, dtype):
        return nc.dram_tensor(name, list(shape), dtype, kind="ExternalInput")

    dr = {}
    for name, arr_shape, dtype in shapes:
        dr[name] = din(name, arr_shape, dtype)
    out_d = nc.dram_tensor("out", [SPC, OUT_D], dt.float32, kind="ExternalOutput")

    # block offsets per (side, cls): column offset of tile t's first block
    boff = {}
    for key, arr in nb.items():
        boff[key] = np.concatenate([[0], np.cumsum(arr)])

    with tile.TileContext(nc) as tc, ExitStack() as ctx:
        nc.gpsimd.load_library(mlp_lib)

        # ---- resident SBUF: constants, indices, outputs of phases -------
        cp = ctx.enter_context(tc.tile_pool(name="const", bufs=1))

        def load_const(name, dtype=None, shape=None):
            h = dr[name]
            shp = shape or list(h.shape)
            t = cp.tile(shp, dtype or h.dtype, tag=name)
            nc.sync.dma_start(t[:], h.ap()[:, :])
            return t

        ramp = load_const("ramp")
        a_bc = load_const("a_bc")
        fcw = load_const("fcw")
        sampid = load_const("sampid")
        xidx = load_const("xidx")
        w = {}
        for d in ("f", "b"):
            w[f"wih_{d}"] = load_const(f"wih_{d}")
            w[f"whh_{d}"] = load_const(f"whh_{d}")
            w[f"bias_{d}"] = load_const(f"bias_{d}")
        ctxid = {}
        sidx = {}
        for side in ("ll", "rl"):
            for cls in ("lo", "hi"):
                ctxid[(side, cls)] = load_const(f"{side}_{cls}_ctx")
                sidx[(side, cls)] = load_const(f"{side}_{cls}_idx")

        big = ctx.enter_context(tc.tile_pool(name="big", bufs=1))
        csp = ctx.enter_context(tc.tile_pool(name="cstage", bufs=6))
        ATT = []
        embT = {s: big.tile([128, NP], dt.bfloat16, tag=f"embT_{s}", name=f"embT_{s}") for s in ("ll", "rl")}
        hT = {d: big.tile([128, NP], dt.bfloat16, tag=f"hT_{d}", name=f"hT_{d}") for d in ("f", "b")}

        # ================= phase A: subtoken segment sums ================
        if 'A' not in phases:
            for s in ('ll','rl'):
                nc.vector.memset(embT[s][:], 0)
        def emit_A(gp, ohp, pap, grps):
            for side in ("ll", "rl"):
                tbl = {"lo": dr["sub_lo"], "hi": dr["sub_hi"]}
                for g in grps:
                    t0, t1 = g * GROUP, min((g + 1) * GROUP, NT128)
                    dst = {}
                    for cls in ("lo", "hi"):
                        b0, b1 = boff[(side, cls)][t0], boff[(side, cls)][t1]
                        nblk = int(b1 - b0)
                        if nblk == 0:
                            continue
                        dtile = gp.tile([128, nblk * 128], dt.bfloat16,
                                        tag=f"g_{cls}", name=f"g_{cls}")
                        d3 = dtile[:].rearrange("p (b e) -> p b e", e=128)
                        for c0 in range(0, nblk, 8):
                            cn = min(8, nblk - c0)
                            nc.gpsimd.dma_gather(
                                d3[:, c0 : c0 + cn, :],
                                tbl[cls].ap()[:, :],
                                sidx[(side, cls)][:, (int(b0) + c0) * 8 : (int(b0) + c0 + cn) * 8],
                                cn * 128,
                                cn * 128,
                                128,
                            )
                        dst[cls] = (dtile, int(b0))
                    for t in range(t0, t1):
                        blocks = []
                        for cls in ("lo", "hi"):
                            nbt = int(nb[(side, cls)][t])
                            if nbt == 0:
                                continue
                            dtile, gb0 = dst[cls]
                            tb0 = int(boff[(side, cls)][t])
                            for j in range(nbt):
                                blocks.append((cls, dtile, tb0 - gb0 + j, tb0 + j))
                        if not blocks:
                            nc.vector.memset(embT[side][:, t * 128 : (t + 1) * 128], 0)
                            continue
                        ps = pap.tile([128, 128], dt.float32, tag="psA", name="psA")
                        for j, (cls, dtile, lb, gcol) in enumerate(blocks):
                            oh = ohp.tile([128, 128], dt.bfloat16, tag="oh", name="oh")
                            nc.vector.tensor_scalar(
                                out=oh[:],
                                in0=ramp[:],
                                scalar1=ctxid[(side, cls)][:, gcol : gcol + 1],
                                scalar2=None,
                                op0=ALU.is_equal,
                            )
                            nc.tensor.matmul(
                                out=ps[:],
                                lhsT=dtile[:, lb * 128 : (lb + 1) * 128],
                                rhs=oh[:],
                                start=(j == 0),
                                stop=(j == len(blocks) - 1),
                            )
                        nc.vector.tensor_copy(
                            out=embT[side][:, t * 128 : (t + 1) * 128], in_=ps[:]
                        )

        # ================= phase B: bidirectional LSTM ===================
        if 'B' not in phases:
            for d in ('f','b'):
                nc.vector.memset(hT[d][:], 0)
        def emit_B(xp, sp, hcp, pbp, spans):
            for (n0, W) in spans:
                xt = xp.tile([128, T * 1024], dt.bfloat16, tag="xt", name="xt")
                x3 = xt[:].rearrange("p (a n) -> p a n", a=1)
                for tg_ in range(T):
                    for h2 in range(0, W, 512):
                        nc.gpsimd.dma_gather(
                            x3[:, :, tg_ * W + h2 : tg_ * W + h2 + 512],
                            dr["node_t"].ap()[:, :],
                            xidx[:, (n0 * 9 + tg_ * W + h2) // 16 :
                                  (n0 * 9 + tg_ * W + h2 + 512) // 16],
                            512,
                            512,
                            128,
                            transpose=True,
                        )
                hp = {"f": None, "b": None}
                cp_ = {"f": None, "b": None}
                for t in range(T):
                    for d in ("f", "b"):
                        h_prev = hp[d]
                        c_prev = cp_[d]
                        tt = t if d == "f" else (T - 1 - t)
                        bias = w[f"bias_{d}"]
                        psg = []
                        for gi in range(4):
                            if t == 0 and gi == 1:
                                psg.append(None)
                                continue
                            pg = pbp.tile([128, 1024], dt.float32, tag="psB",
                                          name="psB")
                            for h2 in range(0, W, 512):
                                nc.tensor.matmul(
                                    out=pg[:, h2 : h2 + 512],
                                    lhsT=w[f"wih_{d}"][:, gi * 128 : (gi + 1) * 128],
                                    rhs=xt[:, tt * W + h2 : tt * W + h2 + 512],
                                    start=True,
                                    stop=(t == 0),
                                )
                                if t > 0:
                                    nc.tensor.matmul(
                                        out=pg[:, h2 : h2 + 512],
                                        lhsT=w[f"whh_{d}"][:, gi * 128 : (gi + 1) * 128],
                                        rhs=h_prev[:, h2 : h2 + 512],
                                        start=False,
                                        stop=True,
                                    )
                            psg.append(pg)
                        # gate order: i, f, g, o
                        si = sp.tile([128, 1024], dt.bfloat16, tag="si", name="si")
                        nc.scalar.activation(si[:, 0:W], psg[0][:, 0:W], AF.Sigmoid,
                                             bias=bias[:, 0:1])
                        if t > 0:
                            sf = sp.tile([128, 1024], dt.bfloat16, tag="sf", name="sf", bufs=2)
                            nc.scalar.activation(sf[:, 0:W], psg[1][:, 0:W], AF.Sigmoid,
                                                 bias=bias[:, 1:2])
                        tg = sp.tile([128, 1024], dt.bfloat16, tag="tg", name="tg", bufs=2)
                        nc.scalar.activation(tg[:, 0:W], psg[2][:, 0:W], AF.Tanh,
                                             bias=bias[:, 2:3])
                        so = sp.tile([128, 1024], dt.bfloat16, tag="so", name="so")
                        nc.scalar.activation(so[:, 0:W], psg[3][:, 0:W], AF.Sigmoid,
                                             bias=bias[:, 3:4])
                        u = sp.tile([128, 1024], dt.bfloat16, tag="u", name="u", bufs=2)
                        nc.vector.tensor_tensor(out=u[:, 0:W], in0=si[:, 0:W],
                                                in1=tg[:, 0:W], op=ALU.mult)
                        if t > 0:
                            v_ = sp.tile([128, 1024], dt.bfloat16, tag="v", name="v", bufs=2)
                            nc.vector.tensor_tensor(out=v_[:, 0:W], in0=sf[:, 0:W],
                                                    in1=c_prev[:, 0:W], op=ALU.mult)
                            c_new = hcp.tile([128, 1024], dt.bfloat16, tag="c", name="c")
                            nc.vector.tensor_tensor(out=c_new[:, 0:W], in0=u[:, 0:W],
                                                    in1=v_[:, 0:W], op=ALU.add)
                        else:
                            c_new = u
                        th = sp.tile([128, 1024], dt.bfloat16, tag="th", name="th", bufs=2)
                        nc.scalar.activation(th[:, 0:W], c_new[:, 0:W], AF.Tanh)
                        if t == T - 1:
                            h_new = None
                            nc.vector.tensor_tensor(
                                out=hT[d][:, n0 : n0 + W],
                                in0=so[:, 0:W], in1=th[:, 0:W], op=ALU.mult)
                        else:
                            h_new = hcp.tile([128, 1024], dt.bfloat16, tag="h", name="h")
                            nc.vector.tensor_tensor(out=h_new[:, 0:W], in0=so[:, 0:W],
                                                    in1=th[:, 0:W], op=ALU.mult)
                        hp[d], cp_[d] = h_new, c_new

        def emit_C(cfp, tiles128):
            feats = [embT["ll"], hT["f"], hT["b"], embT["rl"]]
            for t in tiles128:
                sl = slice(t * 128, (t + 1) * 128)
                psf = cfp.tile([128, DEC], dt.float32, tag="psf", name="psf")
                for gi in range(4):
                    nc.tensor.matmul(
                        out=psf[:],
                        lhsT=feats[gi][:, sl],
                        rhs=fcw[:, gi * DEC : (gi + 1) * DEC],
                        start=(gi == 0),
                        stop=(gi == 3),
                    )
                cxt = csp.tile([128, 322], dt.bfloat16, tag="cxt", name="cxt")
                nc.scalar.activation(cxt[:, 0:DEC], psf[:], AF.Tanh)
                nc.vector.memset(cxt[:, DEC : DEC + 2], 1.0)
                prod = csp.tile([128, DEC], dt.bfloat16, tag="prod", name="prod")
                score = csp.tile([128, 16], dt.float32, tag="score", name="score")
                nc.vector.tensor_tensor(out=prod[:], in0=cxt[:, 0:DEC],
                                        in1=a_bc[:], op=ALU.mult)
                nc.vector.tensor_reduce(out=score[:, 0:1], in_=prod[:],
                                        axis=mybir.AxisListType.X, op=ALU.add)
                ee = csp.tile([128, 16], dt.float32, tag="ee", name="ee")
                nc.scalar.activation(ee[:, 0:1], score[:, 0:1], AF.Exp)
                ectx = csp.tile([128, 322], dt.bfloat16, tag="ectx", name="ectx")
                nc.vector.tensor_scalar(
                    out=ectx[:, 0:322], in0=cxt[:, 0:322], scalar1=ee[:, 0:1],
                    scalar2=None, op0=ALU.mult,
                )
                ohb = csp.tile([128, 65], dt.bfloat16, tag="ohb", name="ohb")
                nc.vector.tensor_scalar(
                    out=ohb[:], in0=ramp[:, 0:65],
                    scalar1=sampid[:, t : t + 1], scalar2=None,
                    op0=ALU.is_equal,
                )
                first, last = (t == 0), (t == NT128 - 1)
                nc.tensor.matmul(out=ATT[0][:, :], lhsT=ectx[:, 0:128], rhs=ohb[:],
                                 start=first, stop=last, skip_group_check=True)
                nc.tensor.matmul(out=ATT[1][:, :], lhsT=ectx[:, 128:256], rhs=ohb[:],
                                 start=first, stop=last, skip_group_check=True)
                nc.tensor.matmul(out=ATT[2][:, :], lhsT=ectx[:, 256:321], rhs=ohb[:],
                                 start=first, stop=last, skip_group_check=True)
                nc.tensor.matmul(out=ATT[3][:, 0:1], lhsT=ohb[:],
                                 rhs=ectx[:, 320:321],
                                 start=first, stop=last, skip_group_check=True)

        with tc.tile_pool(name="gdst", bufs=2) as gp, \
             tc.tile_pool(name="oh", bufs=4) as ohp, \
             tc.tile_pool(name="xt", bufs=2) as xp, \
             tc.tile_pool(name="stage", bufs=3) as sp, \
             tc.tile_pool(name="hc", bufs=2) as hcp, \
             tc.tile_pool(name="psA", bufs=2, space="PSUM") as pap, \
             tc.tile_pool(name="psB", bufs=3, space="PSUM") as pbp:
            for (n0, W) in SPANS:
                if 'A' in phases:
                    emit_A(gp, ohp, pap, range(n0 // 512, (n0 + W) // 512))
                if 'B' in phases:
                    emit_B(xp, sp, hcp, pbp, [(n0, W)])
        if 'C' in phases:
            with tc.tile_pool(name="cps", bufs=4, space="PSUM") as cfp, \
                 tc.tile_pool(name="catt", bufs=1, space="PSUM") as cap:
                ATT.append(cap.tile([128, 65], dt.float32, tag="att0", name="att0"))
                ATT.append(cap.tile([128, 65], dt.float32, tag="att1", name="att1"))
                ATT.append(cap.tile([65, 65], dt.float32, tag="att2", name="att2"))
                ATT.append(cap.tile([65, 16], dt.float32, tag="att3", name="att3"))
                emit_C(cfp, range(NT128))

        # ================= phase C finalize: v, S, output matmul =========
        if 'C' not in phases:
            zz = cp.tile([SPC, OUT_D // 20], dt.float32, tag="zz", name="zz")
            nc.vector.memset(zz[:], 0)
            for j in range(20):
                nc.sync.dma_start(out_d.ap()[:, j*500:(j+1)*500], zz[:])
        else:
            vt = [csp.tile([128, 65], dt.bfloat16, tag=f"vt{c}", name=f"vt{c}") for c in range(2)]
            vt2 = csp.tile([65, 65], dt.bfloat16, tag="vt2", name="vt2")
            nc.vector.tensor_copy(out=vt[0][:], in_=ATT[0][:, :])
            nc.vector.tensor_copy(out=vt[1][:], in_=ATT[1][:, :])
            nc.vector.tensor_copy(out=vt2[:, :], in_=ATT[2][:, :])
            sinv = csp.tile([SPC, 16], dt.float32, tag="sinv", name="sinv")
            nc.vector.reciprocal(sinv[:, 0:1], ATT[3][0:SPC, 0:1])

            with tc.tile_pool(name="wstream", bufs=4) as wp, \
                 tc.tile_pool(name="pso", bufs=2, space="PSUM") as pop:
                nch = OUT_D // OUT_NCH
                for j in range(nch):
                    sl = slice(j * OUT_NCH, (j + 1) * OUT_NCH)
                    r0 = wp.tile([128, OUT_NCH], dt.bfloat16, tag="r0", name="r0")
                    nc.sync.dma_start(r0[:], dr["outw"].ap()[0:128, sl])
                    r1 = wp.tile([128, OUT_NCH], dt.bfloat16, tag="r1", name="r1")
                    nc.sync.dma_start(r1[:], dr["outw"].ap()[128:256, sl])
                    r2 = wp.tile([65, OUT_NCH], dt.bfloat16, tag="r2", name="r2")
                    nc.sync.dma_start(r2[:], dr["outw"].ap()[256:321, sl])
                    po = pop.tile([SPC, OUT_NCH], dt.float32, tag="po", name="po")
                    nc.tensor.matmul(out=po[:], lhsT=vt[0][:, 0:SPC], rhs=r0[:],
                                     start=True, stop=False)
                    nc.tensor.matmul(out=po[:], lhsT=vt[1][:, 0:SPC], rhs=r1[:],
                                     start=False, stop=False)
                    nc.tensor.matmul(out=po[:], lhsT=vt2[:, 0:SPC], rhs=r2[:],
                                     start=False, stop=True)
                    ot = wp.tile([SPC, OUT_NCH], dt.float32, tag="ot", name="ot")
                    nc.vector.tensor_scalar(
                        out=ot[:], in0=po[:], scalar1=sinv[:, 0:1],
                        scalar2=None, op0=ALU.mult,
                    )
                    nc.sync.dma_start(out_d.ap()[:, sl], ot[:])

    nc.compile()
    return nc


# ---- top-level entry ------------------------------------------------------

_CACHE = {}


def _build_in_maps(meta, cores, sh):
    in_maps = []
    for d in cores:
        m = {
            "sub_lo": sh["sub_lo"], "sub_hi": sh["sub_hi"],
            "node_t": sh["node_t"], "ramp": sh["ramp"], "a_bc": sh["a_bc"],
            "fcw": sh["fcw"], "outw": sh["outw"],
            "xidx": d["xidx"], "sampid": d["sampid"],
        }
        for dd in ("f", "b"):
            m[f"wih_{dd}"] = sh[f"wih_{dd}"]
            m[f"whh_{dd}"] = sh[f"whh_{dd}"]
            m[f"bias_{dd}"] = sh[f"bias_{dd}"]
        for side in ("ll", "rl"):
            for cls in ("lo", "hi"):
                m[f"{side}_{cls}_idx"] = d[f"{side}_{cls}_idx"]
                m[f"{side}_{cls}_ctx"] = d[f"{side}_{cls}_ctx"]
        in_maps.append(m)
    return in_maps


def _shapes_of(in_map):
    import concourse.mybir as mybir
    dt = mybir.dt
    np2my = {
        np.dtype(np.float32): dt.float32,
        np.dtype(BF16): dt.bfloat16,
        np.dtype(np.int16): dt.int16,
    }
    return [(k, list(v.shape), np2my[v.dtype]) for k, v in sorted(in_map.items())]


def kernel(**inputs):
    from concourse.bass_utils import run_bass_kernel_spmd

    inp = {k: np.asarray(v) for k, v in inputs.items()}
    meta, cores = prep_all(inp)
    sh = prep_shared(inp)
    in_maps = _build_in_maps(meta, cores, sh)
    shapes = _shapes_of(in_maps[0])
    key = str(shapes) + str({k: v.tolist() for k, v in meta["nb"].items()})
    if key not in _CACHE:
        _CACHE[key] = build_nc(meta, shapes)
    nc = _CACHE[key]
    res = run_bass_kernel_spmd(nc, in_maps, core_ids=list(range(N_CORES)))
    return np.concatenate([res.results[c]["out"] for c in range(N_CORES)], axis=0)

